# revision 17
# baseline (speedup 1.0000x reference)
"""Trainium2 Bass kernel for nn_DeChunkLayer.

Per batch row (one NeuronCore each, pure data parallel):
  1. gate[c]: boundary-sorted clipped probabilities (host, tiny).
  2. EMA linear recurrence over chunks h_c = (1-g_c) h_{c-1} + g_c x_c as a
     blocked lower-triangular matmul "scan": for each 128-chunk block t,
       ema_t = L_t @ X_t (+ lookback term)
     with L entries g_j * prod(1-g_k) host-computed in f64 log space.
     Because the decay product over >=64 chunks underflows far below fp32
     resolution for these gates, each block is computed INDEPENDENTLY from a
     host-verified lookback window of LB preceding chunks (no serial carry
     chain). If the decay bound ever fails, falls back to an exact
     carry-chain formulation (cp (x) h_prev rank-1 matmul per block).
  3. Dechunk out[s] = ema[cid[s]] as one-hot selection matmuls per 128-token
     block; selection matrices built on-device (is_equal vs replicated cid).

All matmul operands are fp16 (PSUM accumulates fp32): values are O(5) so
fp16 keeps abs err ~4e-3 (rel ~3.5e-4) while running the PE at full rate.
ema rows are stored partition-reversed per block so the carry row is
partition 0 (compute engines need 32-aligned partition bases).
"""

import math

import numpy as np

import concourse.bacc as bacc
import concourse.mybir as mybir
from concourse import tile
from concourse.bass_utils import run_bass_kernel_spmd

B, SEQ, MAXC, DIM = 8, 4096, 2048, 1024
BLK = 128
NCORES = 8
NTB = SEQ // BLK  # 32 token blocks
F32 = mybir.dt.float32
F16 = mybir.dt.float16
# output staging group sizes (token blocks per out DMA); tapered tail so the
# final DMA after the last matmul is small
GRPS = [1, 1, 2, 2, 4, 4, 4, 4, 4, 2, 2, 1, 1]
assert sum(GRPS) == NTB


def _preprocess(chunk_states, boundary_mask, boundary_prob):
    """Host-side index/gate math.

    Returns (in_maps, NBLK, windows, LB) where LB>0 selects the lookback
    scan (LB in {64,128}) and LB=0 selects the carry-chain fallback.
    """
    chunk_states = np.asarray(chunk_states, dtype=np.float32)
    boundary_mask = np.asarray(boundary_mask)
    boundary_prob = np.asarray(boundary_prob, dtype=np.float32)

    p_full = np.clip(boundary_prob[..., -1], np.float32(1e-4), np.float32(1.0 - 1e-4))
    token_idx = np.arange(SEQ)[None, :] + (~boundary_mask).astype(np.int32) * SEQ
    order = np.argsort(token_idx, axis=1, kind="stable")
    gate = np.take_along_axis(p_full, order[:, :MAXC], axis=1)  # [B, C]

    cid = np.cumsum(boundary_mask.astype(np.int32), axis=1) - 1  # [B, S]
    cid = np.clip(cid, 0, MAXC - 1)
    n_used = int(cid.max()) + 1
    NBLK = max(1, math.ceil(n_used / BLK))
    CU = NBLK * BLK

    g = gate[:, :CU].astype(np.float64)
    a = 1.0 - g
    S = np.cumsum(np.log(a), axis=1)  # [B, CU] global log-decay prefix

    # pick the smallest lookback window whose dropped prefix is negligible
    LB = 0
    for cand in (64, 128):
        ok = True
        for t in range(1, NBLK):
            j0 = t * BLK - cand - 1
            if j0 < 0:
                continue  # window reaches chunk 0: nothing dropped
            if np.any(S[:, t * BLK] - S[:, j0] > -18.0):
                ok = False
                break
        if ok:
            LB = cand
            break

    ii = np.arange(BLK)[:, None]
    jj = np.arange(BLK)[None, :]
    Sb = S.reshape(B, NBLK, BLK)
    # main (within-block) coefficients: L[b,t,i,j] = g_j exp(S_i - S_j), i>=j
    Lf = np.where(
        ii[None, None] >= jj[None, None],
        np.exp(Sb[:, :, :, None] - Sb[:, :, None, :])
        * g.reshape(B, NBLK, 1, BLK),
        0.0,
    )
    # ema rows stored partition-reversed (chunk i -> partition 127-i)
    Lf = Lf[:, :, ::-1, :]
    LT_sb = np.ascontiguousarray(
        Lf.transpose(0, 3, 1, 2).reshape(B, BLK, NBLK * BLK).astype(np.float16)
    )

    # lookback coefficients: for block t>=1, chunk jb=(t-1)*128+j feeding
    # out chunk t*128+i:  g_jb exp(S[t*128+i] - S[jb]), only j >= 128-LB
    lt2_sb = np.zeros((B, BLK, NBLK * BLK), dtype=np.float16)
    if LB > 0:
        for t in range(1, NBLK):
            Sout = S[:, t * BLK:(t + 1) * BLK]  # [B, 128]
            Sin = S[:, (t - 1) * BLK:t * BLK]  # [B, 128]
            gin = g[:, (t - 1) * BLK:t * BLK]
            Lb = np.exp(Sout[:, None, :] - Sin[:, :, None]) * gin[:, :, None]
            Lb[:, :BLK - LB, :] = 0.0
            # out chunk i -> partition 127-i  => reverse the i axis
            lt2_sb[:, :, t * BLK:(t + 1) * BLK] = Lb[:, :, ::-1].astype(
                np.float16
            )

    # carry-chain fallback data: cp[t,i] = prod_{k<=i in block} a_k, reversed
    ls_blk = np.cumsum(np.log(a).reshape(B, NBLK, BLK), axis=2)
    cp = np.exp(ls_blk).astype(np.float16)[:, :, ::-1]
    cp_sb = np.ascontiguousarray(cp.reshape(B, 1, NBLK * BLK))

    # dechunk union windows per token block
    cidr = cid.reshape(B, NTB, BLK)
    lo = (cidr[:, :, 0] // BLK).min(axis=0)  # [NTB]
    hi = (cidr[:, :, -1] // BLK).max(axis=0)
    windows = [list(range(int(lo[tb]), int(hi[tb]) + 1)) for tb in range(NTB)]
    ncols = sum(len(w) for w in windows)
    jvec = np.empty((BLK, ncols), dtype=np.float32)
    col = 0
    for tb in range(NTB):
        for t in windows[tb]:
            jvec[:, col] = t * BLK + (BLK - 1 - np.arange(BLK))
            col += 1

    in_maps = []
    for b in range(B):
        in_maps.append(
            {
                "x": np.ascontiguousarray(chunk_states[b, :CU].astype(np.float16)),
                "lt": LT_sb[b],
                "lt2": np.ascontiguousarray(lt2_sb[b]),
                "cp": cp_sb[b],
                "cidb": np.ascontiguousarray(
                    np.broadcast_to(cid[b].astype(np.float16)[None, :], (BLK, SEQ))
                ),
                "jvec": jvec,
            }
        )
    return in_maps, NBLK, windows, LB


def _build_nc(NBLK, windows, LB):
    ncols = sum(len(w) for w in windows)
    nc = bacc.Bacc("TRN2", target_bir_lowering=False, debug=False, num_devices=8)
    x = nc.dram_tensor("x", [NBLK * BLK, DIM], F16, kind="ExternalInput")
    lt = nc.dram_tensor("lt", [BLK, NBLK * BLK], F16, kind="ExternalInput")
    lt2 = nc.dram_tensor("lt2", [BLK, NBLK * BLK], F16, kind="ExternalInput")
    cp = nc.dram_tensor("cp", [1, NBLK * BLK], F16, kind="ExternalInput")
    cidb = nc.dram_tensor("cidb", [BLK, SEQ], F16, kind="ExternalInput")
    jvec = nc.dram_tensor("jvec", [BLK, ncols], F32, kind="ExternalInput")
    out = nc.dram_tensor("out", [SEQ, DIM], F16, kind="ExternalOutput")

    with tile.TileContext(nc) as tc:
        with (
            tc.tile_pool(name="const", bufs=1) as const_pool,
            tc.tile_pool(name="selp", bufs=8) as selpool,
            tc.tile_pool(name="outp", bufs=3) as outpool,
            tc.tile_pool(name="ps_scan", bufs=1, space="PSUM") as ps_scan,
            tc.tile_pool(name="ps_out", bufs=3, space="PSUM") as ps_out,
        ):
            # load order: scan weights + x first (critical path), index data
            # for the dechunk on the second HWDGE ring.
            lt_sb = const_pool.tile([BLK, NBLK * BLK], F16, tag="lt")
            nc.sync.dma_start(lt_sb[:], lt[:])
            x_sb = const_pool.tile([BLK, NBLK * DIM], F16, tag="x")
            # stage x in up to three pieces so early scan blocks unblock fast
            xcuts = sorted({min(2, NBLK), min(5, NBLK), NBLK})
            nc.sync.dma_start(
                x_sb[:, :xcuts[0] * DIM],
                x[0:xcuts[0] * BLK, :].rearrange("(t p) d -> p t d", p=BLK),
            )
            lt2_sb = None
            if LB > 0:
                lt2_sb = const_pool.tile([BLK, NBLK * BLK], F16, tag="lt2")
                nc.sync.dma_start(lt2_sb[:], lt2[:])
            cp_sb = const_pool.tile([1, NBLK * BLK], F16, tag="cp")
            if LB == 0:
                nc.sync.dma_start(cp_sb[:], cp[:])
            for c0, c1 in zip(xcuts, xcuts[1:]):
                nc.sync.dma_start(
                    x_sb[:, c0 * DIM:c1 * DIM],
                    x[c0 * BLK:c1 * BLK, :].rearrange("(t p) d -> p t d", p=BLK),
                )
            cidb_sb = const_pool.tile([BLK, SEQ], F16, tag="cidb")
            nc.scalar.dma_start(cidb_sb[:, :1024], cidb[:, :1024])
            nc.scalar.dma_start(cidb_sb[:, 1024:], cidb[:, 1024:])
            jvec_sb = const_pool.tile([BLK, ncols], F32, tag="jvec")
            nc.scalar.dma_start(jvec_sb[:], jvec[:])
            ema = const_pool.tile([BLK, NBLK * DIM], F16, tag="ema")

            # PE warmup: zero-weight matmuls accumulating into block 0's
            # psum (add 0, cannot be dead-code-eliminated). ~4us of PE
            # activity releases the HAM clock throttle before real work.
            zw = const_pool.tile([BLK, BLK], F16, tag="zw")
            nc.gpsimd.memset(zw[:], 0.0)
            zx = const_pool.tile([BLK, 512], F16, tag="zx")
            nc.gpsimd.memset(zx[:], 0.0)
            ps0 = ps_scan.tile([BLK, DIM], F32, tag="ps")
            for k in range(4):
                for h in range(2):
                    nc.tensor.matmul(
                        ps0[:, h * 512:(h + 1) * 512], lhsT=zw[:], rhs=zx[:],
                        start=(k == 0), stop=False,
                    )

            # ---- dechunk emitter (interleaved with the scan so the PE
            # queue never stalls behind scan blocks waiting on late DMAs) ---
            state = {"col": 0, "tb": 0, "gi": 0}

            def emit_group(grp):
                gi = state["gi"]
                og = outpool.tile([BLK, grp * DIM], F16, tag=f"og{grp}",
                                  name=f"og_{gi}")
                for i in range(grp):
                    tb = state["tb"]
                    col = state["col"]
                    w = windows[tb]
                    sels = []
                    for t in w:
                        sel = selpool.tile([BLK, BLK], F16, tag="sel",
                                           name=f"sel_{col}")
                        nc.vector.tensor_scalar(
                            out=sel[:],
                            in0=cidb_sb[:, tb * BLK:(tb + 1) * BLK],
                            scalar1=jvec_sb[:, col:col + 1],
                            scalar2=None,
                            op0=mybir.AluOpType.is_equal,
                        )
                        sels.append((sel, t))
                        col += 1
                    state["col"] = col
                    po = ps_out.tile([BLK, DIM], F32, tag="po",
                                     name=f"po_{tb}")
                    for wi, (sel, t) in enumerate(sels):
                        for h in range(2):
                            nc.tensor.matmul(
                                po[:, h * 512:(h + 1) * 512],
                                lhsT=sel[:],
                                rhs=ema[:, t * DIM + h * 512:
                                        t * DIM + (h + 1) * 512],
                                start=(wi == 0),
                                stop=(wi == len(sels) - 1),
                            )
                    dst = og[:, i * DIM:(i + 1) * DIM]
                    if (tb % 2) == 0:
                        nc.scalar.copy(out=dst, in_=po[:])
                    else:
                        nc.vector.tensor_copy(out=dst, in_=po[:])
                    state["tb"] = tb + 1
                tb0 = state["tb"] - grp
                dma_eng = nc.sync if (gi % 2) == 0 else nc.scalar
                dma_eng.dma_start(
                    out[tb0 * BLK:state["tb"] * BLK, :].rearrange(
                        "(i p) d -> p i d", p=BLK
                    ),
                    og[:].rearrange("p (i d) -> p i d", d=DIM),
                )
                state["gi"] = gi + 1

            # a group is ready once the last ema block it reads is written
            group_need = []
            tb = 0
            for grp in GRPS:
                group_need.append(max(max(windows[t]) for t in range(tb, tb + grp)))
                tb += grp

            # ---- blocked matmul scan over chunk blocks ----
            for t in range(NBLK):
                ps = ps0 if t == 0 else ps_scan.tile([BLK, DIM], F32, tag="ps")
                for h in range(2):
                    sl = slice(h * 512, (h + 1) * 512)
                    xsl = slice(t * DIM + h * 512, t * DIM + (h + 1) * 512)
                    nc.tensor.matmul(
                        ps[:, sl],
                        lhsT=lt_sb[:, t * BLK:(t + 1) * BLK],
                        rhs=x_sb[:, xsl],
                        start=(t != 0),
                        stop=(t == 0),
                    )
                    if t > 0:
                        if LB > 0:
                            p0 = BLK - LB
                            lsl = slice((t - 1) * DIM + h * 512,
                                        (t - 1) * DIM + (h + 1) * 512)
                            nc.tensor.matmul(
                                ps[:, sl],
                                lhsT=lt2_sb[p0:BLK, t * BLK:(t + 1) * BLK],
                                rhs=x_sb[p0:BLK, lsl],
                                start=False,
                                stop=True,
                            )
                        else:
                            # carry chain: cp_t (x) h_prev, h_prev = row 0 of
                            # the previous block's (reversed) fp16 ema
                            esl = slice((t - 1) * DIM + h * 512,
                                        (t - 1) * DIM + (h + 1) * 512)
                            nc.tensor.matmul(
                                ps[:, sl],
                                lhsT=cp_sb[:, t * BLK:(t + 1) * BLK],
                                rhs=ema[0:1, esl],
                                start=False,
                                stop=True,
                            )
                # psum -> fp16 ema, split across DVE and ACT
                nc.vector.tensor_copy(
                    out=ema[:, t * DIM:t * DIM + 512], in_=ps[:, :512]
                )
                nc.scalar.copy(
                    out=ema[:, t * DIM + 512:(t + 1) * DIM], in_=ps[:, 512:]
                )
                while (state["gi"] < len(GRPS)
                       and group_need[state["gi"]] <= t):
                    emit_group(GRPS[state["gi"]])

            while state["gi"] < len(GRPS):
                emit_group(GRPS[state["gi"]])

    nc.finalize()
    return nc


def _run(in_maps, NBLK, windows, LB):
    nc = _build_nc(NBLK, windows, LB)
    res = run_bass_kernel_spmd(nc, in_maps, core_ids=list(range(NCORES)))
    return np.stack(
        [res.results[i]["out"].astype(np.float32) for i in range(NCORES)], axis=0
    )


def kernel(chunk_states, boundary_mask, boundary_prob):
    in_maps, NBLK, windows, LB = _preprocess(
        chunk_states, boundary_mask, boundary_prob
    )
    last_err = None
    for _ in range(3):  # retry transient accelerator failures
        try:
            return _run(in_maps, NBLK, windows, LB)
        except Exception as e:  # noqa: BLE001
            last_err = e
            try:
                import jax

                jax.clear_caches()
            except Exception:  # noqa: BLE001
                pass
    raise last_err
